# revision 1
# baseline (speedup 1.0000x reference)
"""Trainium2 Bass kernel: teacher-forced LSTM decoder + packed vocab projection.

Model (B=128, T=20, E=H=512, V=32000):
  x = [features, embed(captions[:, :T-1])]            # [B, T, E]
  (h, c) LSTM-scan over T steps (PyTorch gate order i,f,g,o)
  logits = hs @ lin_w.T + lin_b                       # [T, B, V]
  out = logits packed time-major, keeping rows with length > t  # [sum(len), V]

Strategy (8 NeuronCores):
  - Vocab-parallel: core s owns lin_w columns [s*4000, (s+1)*4000).
  - The LSTM recurrence is replicated on every core (it is serial over T and
    every core needs all hidden states for its vocab slice).
  - Host prep: embedding gather + packing into xT [E, L] (L = sum(lengths),
    only rows that survive packing), weight transposes, per-core lin_w shard.
  - On device, per step t (n_t = #rows with length > t, descending lengths):
      gates[n_t, 4H] accumulate in PSUM: xT-part matmul + h-part matmul (fp32r)
      sigmoid/tanh on ScalarE, c/h update on VectorE,
      h transposed back to [H, n_t] via PE transpose -> feeds next step's
      matmul and the projection stash hs[:, L].
  - Projection phase: for each 500-wide vocab slice, matmul all L packed rows
    against lin_w.T slice (fp32r), copy PSUM->SBUF, DMA to out [L, 4000].
  - Host gathers the 8 core outputs and concatenates along vocab.
"""

import math

import numpy as np

import concourse.bacc as bacc
import concourse.bass as bass
import concourse.mybir as mybir
import concourse.tile as tile
from concourse.bass_utils import run_bass_kernel_spmd

B, T, E, H, V = 128, 20, 512, 512, 32000
NCORES = 8
VS = V // NCORES      # per-core vocab shard (4000)
NV = 8                # vocab sub-chunks per core
VC = VS // NV         # 500 columns per projection matmul (>=256 keeps fp32r fast)
KE = E // 128         # 4 contraction chunks over E
KH = H // 128         # 4 contraction chunks over H
P = 128

F32 = mybir.dt.float32
F32R = mybir.dt.float32r
AF = mybir.ActivationFunctionType


def build_program(n_ts, use_bias, use_linb):
    """Build the single-core Bass/Tile program (same program on all 8 cores).

    n_ts: per-step active-row counts (descending, all > 0), len(n_ts) <= T.
    """
    L = int(sum(n_ts))
    offs = np.concatenate([[0], np.cumsum(n_ts)]).astype(int)
    nchunks = math.ceil(L / P)

    nc = bacc.Bacc("TRN2", target_bir_lowering=False, debug=False)

    xT_d = nc.dram_tensor("xT", [E, L], F32R, kind="ExternalInput")
    wih_d = nc.dram_tensor("wih", [E, 4 * H], F32R, kind="ExternalInput")
    whh_d = nc.dram_tensor("whh", [H, 4 * H], F32R, kind="ExternalInput")
    h0T_d = nc.dram_tensor("h0T", [H, B], F32R, kind="ExternalInput")
    c0_d = nc.dram_tensor("c0", [B, H], F32, kind="ExternalInput")
    id_d = nc.dram_tensor("ident", [P, P], F32R, kind="ExternalInput")
    linT_d = nc.dram_tensor("linT", [H, VS], F32R, kind="ExternalInput")
    bias_d = linb_d = None
    if use_bias:
        bias_d = nc.dram_tensor("bias2", [1, 4 * H], F32R, kind="ExternalInput")
    if use_linb:
        linb_d = nc.dram_tensor("linb", [1, VS], F32R, kind="ExternalInput")
    out_d = nc.dram_tensor("out", [L, VS], F32, kind="ExternalOutput")

    PS = bass.MemorySpace.PSUM

    with tile.TileContext(nc) as tc:
        with (
            tc.tile_pool(name="persist", bufs=1) as pers,
            tc.tile_pool(name="xt", bufs=4) as xtp,
            tc.tile_pool(name="hT", bufs=2) as hTp,
            tc.tile_pool(name="cc", bufs=2) as ccp,
            tc.tile_pool(name="work", bufs=10) as wkp,
            tc.tile_pool(name="lint", bufs=5) as ltp,
            tc.tile_pool(name="outs", bufs=6) as otp,
            tc.tile_pool(name="gps", bufs=4, space=PS) as gpsp,
            tc.tile_pool(name="tps", bufs=1, space=PS) as tpsp,
            tc.tile_pool(name="pps", bufs=3, space=PS) as ppsp,
        ):
            # ---- identity + HAM warmup (PE would otherwise idle cold) ----
            ident = pers.tile([P, P], F32R, tag="ident")
            nc.sync.dma_start(ident[:], id_d[:])
            warm = tpsp.tile([P, 512], F32, tag="tp", name="warm")
            for w in range(32):
                nc.tensor.matmul(
                    warm[:, 128 * (w % 4) : 128 * (w % 4 + 1)],
                    ident[:], ident[:], start=True, stop=True,
                )

            # ---- initial state + first-step inputs (tiny, consumed first) ----
            hT_prev = hTp.tile([P, KH, P], F32R, tag="hT", name="hT0")
            for k in range(KH):
                nc.sync.dma_start(hT_prev[:, k, :], h0T_d[P * k : P * (k + 1), :])
            c_prev = ccp.tile([P, H], F32, tag="c", name="c0t")
            nc.sync.dma_start(c_prev[:], c0_d[:])

            xt_tiles = {}

            def prefetch_xt(t):
                if t >= len(n_ts):
                    return
                n = int(n_ts[t])
                off = int(offs[t])
                xt = xtp.tile([P, KE, P], F32R, tag="xt", name=f"xt{t}")
                for k in range(KE):
                    nc.sync.dma_start(
                        xt[:, k, :n], xT_d[P * k : P * (k + 1), off : off + n]
                    )
                xt_tiles[t] = xt

            prefetch_xt(0)
            prefetch_xt(1)
            prefetch_xt(2)

            # ---- resident weights, j-major to match consumption order ----
            wih = [pers.tile([P, 4, 512], F32R, tag=f"wih{k}", name=f"wih{k}") for k in range(KE)]
            whh = [pers.tile([P, 4, 512], F32R, tag=f"whh{k}", name=f"whh{k}") for k in range(KH)]
            for j in range(4):
                for k in range(KE):
                    nc.sync.dma_start(
                        wih[k][:, j, :], wih_d[P * k : P * (k + 1), 512 * j : 512 * (j + 1)]
                    )
                for k in range(KH):
                    nc.sync.dma_start(
                        whh[k][:, j, :], whh_d[P * k : P * (k + 1), 512 * j : 512 * (j + 1)]
                    )
            bias_t = ones_t = linb_t = None
            if use_bias or use_linb:
                ones_t = pers.tile([1, P], F32R, tag="ones")
                nc.vector.memset(ones_t[:], 1.0)
            if use_bias:
                bias_t = pers.tile([1, 4 * H], F32R, tag="bias2")
                nc.sync.dma_start(bias_t[:], bias_d[:])
            if use_linb:
                linb_t = pers.tile([1, VS], F32R, tag="linb")
                nc.sync.dma_start(linb_t[:], linb_d[:])

            # packed transposed hidden states: hs[c][k] is [128, 128] holding
            # hsT[128k:128(k+1), 128c:128(c+1)] of the [H, L] stash.
            hs = [
                [pers.tile([P, P], F32R, tag=f"hs{c}_{k}", name=f"hs{c}_{k}") for k in range(KH)]
                for c in range(nchunks)
            ]

            # ---- projection helpers: first NVPRE vocab phases interleave ----
            NVPRE = 3
            lt_tiles = {}
            for v in range(NVPRE):
                lt = ltp.tile([P, KH, VC], F32R, tag="lt", name=f"lt{v}")
                for k in range(KH):
                    nc.sync.dma_start(
                        lt[:, k, :], linT_d[P * k : P * (k + 1), VC * v : VC * (v + 1)]
                    )
                lt_tiles[v] = lt

            def emit_proj(c, v, lt, interleaved=False):
                mc = min(P, L - P * c)
                vs = slice(VC * v, VC * (v + 1))
                pp = ppsp.tile([P, VC], F32, tag="pp", name="pp")
                for k in range(KH):
                    nc.tensor.matmul(
                        pp[:mc, :],
                        hs[c][k][:, :mc],
                        lt[:, k, :],
                        start=(k == 0),
                        stop=(k == KH - 1) and not use_linb,
                    )
                if use_linb:
                    nc.tensor.matmul(
                        pp[:mc, :], ones_t[:1, :mc], linb_t[:1, vs],
                        start=False, stop=True,
                    )
                ot = otp.tile([P, VC], F32, tag="ot", name="ot")
                if interleaved or (c + v) % 2 == 1:
                    nc.vector.tensor_copy(ot[:mc, :], pp[:mc, :])
                else:
                    nc.scalar.copy(ot[:mc, :], pp[:mc, :])
                nc.sync.dma_start(out_d[P * c : P * c + mc, vs], ot[:mc, :])

            done_chunks = 0
            INTERLEAVE_FROM = 2  # first chunks deferred: their lt may not be loaded yet
            deferred = []

            # ---- recurrence over packed steps ----
            for t, n in enumerate(n_ts):
                n = int(n)
                off = int(offs[t])
                n_next = int(n_ts[t + 1]) if t + 1 < len(n_ts) else 0

                xt = xt_tiles.pop(t)
                prefetch_xt(t + 3)

                # gates = x_t @ w_ih.T + h_{t-1} @ w_hh.T (+ bias), PSUM accum
                g_ps = []
                for j in range(4):
                    g = gpsp.tile([P, 512], F32, tag="g", name="g")
                    js = slice(512 * j, 512 * (j + 1))
                    for k in range(KE):
                        nc.tensor.matmul(
                            g[:n, :], xt[:, k, :n], wih[k][:, j, :],
                            start=(k == 0), stop=False,
                        )
                    if use_bias:
                        nc.tensor.matmul(
                            g[:n, :], ones_t[:1, :n], bias_t[:1, js],
                            start=False, stop=False,
                        )
                    for k in range(KH):
                        nc.tensor.matmul(
                            g[:n, :], hT_prev[:, k, :n], whh[k][:, j, :],
                            start=False, stop=(k == KH - 1),
                        )
                    g_ps.append(g)

                # nonlinearities: i, f, g, o -> c, h
                i_s = wkp.tile([P, 512], F32, tag="wk")
                f_s = wkp.tile([P, 512], F32, tag="wk")
                g_s = wkp.tile([P, 512], F32, tag="wk")
                o_s = wkp.tile([P, 512], F32, tag="wk")
                nc.scalar.activation(i_s[:n, :], g_ps[0][:n, :], AF.Sigmoid)
                nc.scalar.activation(f_s[:n, :], g_ps[1][:n, :], AF.Sigmoid)
                nc.scalar.activation(g_s[:n, :], g_ps[2][:n, :], AF.Tanh)
                nc.scalar.activation(o_s[:n, :], g_ps[3][:n, :], AF.Sigmoid)
                t1 = wkp.tile([P, 512], F32, tag="wk")
                t2 = wkp.tile([P, 512], F32, tag="wk")
                nc.vector.tensor_mul(t1[:n, :], i_s[:n, :], g_s[:n, :])
                nc.vector.tensor_mul(t2[:n, :], f_s[:n, :], c_prev[:n, :])
                c_new = ccp.tile([P, H], F32, tag="c", name="cn")
                nc.vector.tensor_add(c_new[:n, :], t1[:n, :], t2[:n, :])
                tct = wkp.tile([P, 512], F32, tag="wk")
                nc.scalar.activation(tct[:n, :], c_new[:n, :], AF.Tanh)
                h_sb = wkp.tile([P, 512], F32R, tag="wkh", bufs=3)
                nc.vector.tensor_mul(h_sb[:n, :], o_s[:n, :], tct[:n, :])

                # transpose h back to [H, n] (PE transpose, one PSUM bank)
                psT = tpsp.tile([P, 512], F32R, tag="tp", name="psT")
                for k in range(KH):
                    nc.tensor.transpose(
                        psT[:, P * k : P * k + n],
                        h_sb[:n, P * k : P * (k + 1)],
                        ident[:n, :n],
                    )

                # hT state for the next step (critical path: emit first)
                if n_next > 0:
                    hT_new = hTp.tile([P, KH, P], F32R, tag="hT", name="hTn")
                    for k in range(KH):
                        nc.vector.tensor_copy(
                            hT_new[:, k, :n_next], psT[:, P * k : P * k + n_next]
                        )
                    hT_prev = hT_new
                c_prev = c_new

                # stash into hs chunks (cols [off, off+n) of the [H, L] stash)
                c0i = off // P
                c1i = (off + n - 1) // P
                for k in range(KH):
                    if c0i == c1i:
                        nc.vector.tensor_copy(
                            hs[c0i][k][:, off - P * c0i : off - P * c0i + n],
                            psT[:, P * k : P * k + n],
                        )
                    else:
                        a = P * c1i - off
                        nc.vector.tensor_copy(
                            hs[c0i][k][:, off - P * c0i :], psT[:, P * k : P * k + a]
                        )
                        nc.vector.tensor_copy(
                            hs[c1i][k][:, : n - a],
                            psT[:, P * k + a : P * k + n],
                        )

                # interleave projection for fully-completed 128-row chunks
                while (done_chunks + 1) * P <= int(offs[t + 1]):
                    if done_chunks < INTERLEAVE_FROM:
                        deferred.append(done_chunks)
                    else:
                        for v in range(NVPRE):
                            emit_proj(done_chunks, v, lt_tiles[v], interleaved=True)
                    done_chunks += 1

            # ---- projection: remaining work ----
            for c in deferred + list(range(done_chunks, nchunks)):
                for v in range(NVPRE):
                    emit_proj(c, v, lt_tiles[v])
            for v in range(NVPRE, NV):
                lt = ltp.tile([P, KH, VC], F32R, tag="lt", name=f"lt{v}")
                for k in range(KH):
                    nc.sync.dma_start(
                        lt[:, k, :], linT_d[P * k : P * (k + 1), VC * v : VC * (v + 1)]
                    )
                for c in range(nchunks):
                    emit_proj(c, v, lt)

    nc.compile()
    return nc


_prog_cache = {}


def _get_program(n_ts, use_bias, use_linb):
    key = (tuple(int(x) for x in n_ts), bool(use_bias), bool(use_linb))
    if key not in _prog_cache:
        _prog_cache[key] = build_program(n_ts, use_bias, use_linb)
    return _prog_cache[key]


def kernel(
    features,
    captions,
    lengths,
    h0,
    c0,
    embed_w,
    w_ih,
    w_hh,
    b_ih,
    b_hh,
    lin_w,
    lin_b,
    maxlen,
    _trace=False,
):
    features = np.asarray(features, np.float32)
    captions = np.asarray(captions)
    lengths = np.asarray(lengths)
    h0 = np.asarray(h0, np.float32)
    c0 = np.asarray(c0, np.float32)
    embed_w = np.asarray(embed_w, np.float32)
    w_ih = np.asarray(w_ih, np.float32)
    w_hh = np.asarray(w_hh, np.float32)
    b_ih = np.asarray(b_ih, np.float32)
    b_hh = np.asarray(b_hh, np.float32)
    lin_w = np.asarray(lin_w, np.float32)
    lin_b = np.asarray(lin_b, np.float32)
    maxlen = int(maxlen)
    batch = captions.shape[0]

    # Sort rows by descending length (stable). pack_padded_sequence requires
    # descending lengths, so perm is normally the identity; the permutation
    # fallback keeps us correct on arbitrary length order.
    ln = lengths.astype(np.int64)
    perm = np.argsort(-ln, kind="stable")
    identity_perm = bool(np.all(perm == np.arange(batch)))
    lns = ln[perm]

    n_ts = []
    for t in range(maxlen):
        n = int((lns > t).sum())
        if n == 0:
            break
        n_ts.append(n)
    L = int(sum(n_ts))
    offs = np.concatenate([[0], np.cumsum(n_ts)]).astype(int)

    # host prep: packed transposed input sequence xT [E, L]
    xs = np.empty((L, E), np.float32)
    for t, n in enumerate(n_ts):
        sel = perm[:n]
        if t == 0:
            xs[offs[t] : offs[t] + n] = features[sel]
        else:
            xs[offs[t] : offs[t] + n] = embed_w[captions[sel, t - 1]]
    xT = np.ascontiguousarray(xs.T)

    wihT = np.ascontiguousarray(w_ih.T)
    whhT = np.ascontiguousarray(w_hh.T)
    h0T = np.ascontiguousarray(h0[perm].T)
    c0p = np.ascontiguousarray(c0[perm])
    linT = np.ascontiguousarray(lin_w.T)
    ident = np.eye(P, dtype=np.float32)
    bias2 = (b_ih + b_hh).astype(np.float32)
    use_bias = bool(np.any(bias2))
    use_linb = bool(np.any(lin_b))

    nc = _get_program(n_ts, use_bias, use_linb)

    in_maps = []
    for s in range(NCORES):
        m = {
            "xT": xT,
            "wih": wihT,
            "whh": whhT,
            "h0T": h0T,
            "c0": c0p,
            "ident": ident,
            "linT": np.ascontiguousarray(linT[:, VS * s : VS * (s + 1)]),
        }
        if use_bias:
            m["bias2"] = bias2.reshape(1, 4 * H)
        if use_linb:
            m["linb"] = np.ascontiguousarray(
                lin_b[VS * s : VS * (s + 1)].reshape(1, VS).astype(np.float32)
            )
        in_maps.append(m)

    res = run_bass_kernel_spmd(
        nc, in_maps, core_ids=list(range(NCORES)), trace=_trace
    )
    out = np.concatenate([np.asarray(r["out"]) for r in res.results], axis=1)

    if not identity_perm:
        # map packed rows computed in sorted order back to original order
        src = np.empty(L, np.int64)
        pos = 0
        inv_pos = {}
        for t, n in enumerate(n_ts):
            for j in range(n):
                inv_pos[(t, int(perm[j]))] = offs[t] + j
        for t in range(maxlen):
            for i in np.nonzero(ln > t)[0]:
                src[pos] = inv_pos[(t, int(i))]
                pos += 1
        out = out[src]

    if _trace:
        return out, res
    return out



# revision 2
# speedup vs baseline: 1.4574x; 1.4574x over previous
"""Trainium2 Bass kernel: teacher-forced LSTM decoder + packed vocab projection.

Model (B=128, T=20, E=H=512, V=32000):
  x = [features, embed(captions[:, :T-1])]            # [B, T, E]
  (h, c) LSTM-scan over T steps (PyTorch gate order i,f,g,o)
  logits = hs @ lin_w.T + lin_b                       # [T, B, V]
  out = logits packed time-major, keeping rows with length > t  # [sum(len), V]

Strategy (8 NeuronCores, v2):
  - Vocab-parallel: core s owns lin_w columns [s*4000, (s+1)*4000).
  - All matmuls bf16 (PE streams ~0.52 ns/col regardless of bf16/fp32r, but
    bf16 keeps the LDWEIGHTS load hidden and halves weight DMA).
  - Everything is computed in TRANSPOSED orientation ([feature, row] with
    features on partitions), so gate matmuls stream only n_t columns per
    step (exploiting packed shrinking batches) and no PE transposes are
    needed anywhere: h comes out of the LSTM already as hsT [H, L].
  - x-part of the gates is batched over all L packed rows up front:
    xWT [4H, L] = wih.T-chunks @ xT, stashed bf16 (+bias folded in evac).
  - Per step t: 64 matmuls accumulate whh.T-chunks @ hT_{t-1} into 4 PSUM
    banks [128, 4, n] (bank j = gate j); DVE adds the xWT slice; ScalarE
    sigmoid/tanh; DVE c/h update writes h directly into the bf16 hsT stash
    (which doubles as next step's matmul moving operand and the projection's
    stationary operand).
  - The PE queue is a hand-scheduled FIFO: x-units / projection units are
    interleaved between recurrence steps as fillers so the PE never idles
    during the serial per-step activation chain.
  - Projection: for each 128-row L-chunk x 500-wide vocab slice, 4 bf16
    matmuls (hsT chunk stationary, linT moving) -> PSUM -> SBUF -> DMA out.
  - Host gathers the 8 core outputs and concatenates along vocab.
"""

import math
from collections import deque

import numpy as np
import ml_dtypes

import concourse.bacc as bacc
import concourse.bass as bass
import concourse.mybir as mybir
import concourse.tile as tile
from concourse.bass_utils import run_bass_kernel_spmd

B, T, E, H, V = 128, 20, 512, 512, 32000
NCORES = 8
VS = V // NCORES      # per-core vocab shard (4000)
NV = 8                # vocab sub-chunks per core
VC = VS // NV         # 500 columns per projection matmul
KE = E // 128         # 4 contraction chunks over E
KH = H // 128         # 4 contraction chunks over H
P = 128
G4 = 4 * H            # 2048 gate dims
NM = G4 // P          # 16 gate M-chunks

F32 = mybir.dt.float32
BF16 = mybir.dt.bfloat16
AF = mybir.ActivationFunctionType

BF = ml_dtypes.bfloat16

FILL_NS = 3400        # PE filler budget per recurrence gap (hides act chain)


def build_program(n_ts, use_bias, use_linb):
    """Single-core Bass/Tile program (same program on all 8 cores)."""
    L = int(sum(n_ts))
    offs = np.concatenate([[0], np.cumsum(n_ts)]).astype(int)
    nchunks = math.ceil(L / P)
    # L-chunks of 512 for the batched x-part
    lcs = [(c0, min(512, L - c0)) for c0 in range(0, L, 512)]

    nc = bacc.Bacc("TRN2", target_bir_lowering=False, debug=False)

    xT_d = nc.dram_tensor("xT", [E, L], BF16, kind="ExternalInput")
    wih_d = nc.dram_tensor("wih", [E, G4], BF16, kind="ExternalInput")
    whh_d = nc.dram_tensor("whh", [H, G4], BF16, kind="ExternalInput")
    h0T_d = nc.dram_tensor("h0T", [H, B], BF16, kind="ExternalInput")
    c0T_d = nc.dram_tensor("c0T", [H, B], F32, kind="ExternalInput")
    linT_d = nc.dram_tensor("linT", [H, VS], BF16, kind="ExternalInput")
    bias_d = linb_d = None
    if use_bias:
        bias_d = nc.dram_tensor("biaspm", [P, NM], F32, kind="ExternalInput")
    if use_linb:
        linb_d = nc.dram_tensor("linb", [1, VS], BF16, kind="ExternalInput")
    out_d = nc.dram_tensor("out", [L, VS], F32, kind="ExternalOutput")

    PS = bass.MemorySpace.PSUM

    with tile.TileContext(nc) as tc:
        with (
            tc.tile_pool(name="persist", bufs=1) as pers,
            tc.tile_pool(name="cc", bufs=2) as ccp,
            tc.tile_pool(name="work", bufs=12) as wkp,
            tc.tile_pool(name="outs", bufs=6) as otp,
            tc.tile_pool(name="gps", bufs=4, space=PS) as gpsp,
            tc.tile_pool(name="xps", bufs=2, space=PS) as xpsp,
            tc.tile_pool(name="pps", bufs=2, space=PS) as ppsp,
        ):
            # ---- resident tensors ----
            wih_sb = pers.tile([P, KE, NM, P], BF16, tag="wih")
            xT_sb = pers.tile([P, KE, L], BF16, tag="xT")
            h0_sb = pers.tile([P, KH, P], BF16, tag="h0")
            c0_sb = ccp.tile([P, KH, P], F32, tag="c", name="c0t")
            whh_sb = pers.tile([P, KH, NM, P], BF16, tag="whh")
            lin_sb = pers.tile([P, KH, VS], BF16, tag="lin")
            xWT = pers.tile([P, NM, L], BF16, tag="xWT")
            hsT = pers.tile([P, KH, L], BF16, tag="hsT")

            # DMA order = need order: wih + xT head -> rec state -> whh -> lin
            for k in range(KE):
                nc.sync.dma_start(wih_sb[:, k, :, :], wih_d[P * k : P * (k + 1), :])
            nc.sync.dma_start(xT_sb[:, :, : lcs[0][1]].opt(),
                              xT_d[:, : lcs[0][1]].rearrange("(k p) l -> p k l", k=KE))
            for k in range(KH):
                nc.sync.dma_start(h0_sb[:, k, :], h0T_d[P * k : P * (k + 1), :])
            for k in range(KH):
                nc.sync.dma_start(c0_sb[:, k, :], c0T_d[P * k : P * (k + 1), :])
            for k in range(KH):
                nc.sync.dma_start(whh_sb[:, k, :, :], whh_d[P * k : P * (k + 1), :])
            if lcs[1:]:
                c0 = lcs[1][0]
                nc.sync.dma_start(xT_sb[:, :, c0:].opt(),
                                  xT_d[:, c0:].rearrange("(k p) l -> p k l", k=KE))
            bias_sb = ones_t = linb_sb = None
            if use_bias:
                bias_sb = pers.tile([P, NM], F32, tag="bias")
                nc.sync.dma_start(bias_sb[:], bias_d[:])
            if use_linb:
                ones_t = pers.tile([1, P], BF16, tag="ones")
                nc.vector.memset(ones_t[:], 1.0)
                linb_sb = pers.tile([1, VS], BF16, tag="linb")
                nc.sync.dma_start(linb_sb[:], linb_d[:])
            for k in range(KH):
                nc.sync.dma_start(lin_sb[:, k, :], linT_d[P * k : P * (k + 1), :])

            # ---- filler units (run on the PE between recurrence steps) ----
            def emit_x_unit(m, ci):
                c0, w = lcs[ci]
                xp = xpsp.tile([P, 512], F32, tag="xp", name="xp")
                for k in range(KE):
                    nc.tensor.matmul(
                        xp[:, :w], wih_sb[:, k, m, :], xT_sb[:, k, c0 : c0 + w],
                        start=(k == 0), stop=(k == KE - 1),
                    )
                if use_bias:
                    nc.vector.tensor_scalar_add(
                        xWT[:, m, c0 : c0 + w], xp[:, :w], bias_sb[:, m : m + 1]
                    )
                else:
                    nc.vector.tensor_copy(xWT[:, m, c0 : c0 + w], xp[:, :w])

            def emit_proj(c, v, alt):
                mc = min(P, L - P * c)
                vs = slice(VC * v, VC * (v + 1))
                pp = ppsp.tile([P, VC], F32, tag="pp", name="pp")
                for k in range(KH):
                    nc.tensor.matmul(
                        pp[:mc, :], hsT[:, k, P * c : P * c + mc], lin_sb[:, k, vs],
                        start=(k == 0), stop=(k == KH - 1) and not use_linb,
                    )
                if use_linb:
                    nc.tensor.matmul(
                        pp[:mc, :], ones_t[:1, :mc], linb_sb[:1, vs],
                        start=False, stop=True,
                    )
                ot = otp.tile([P, VC], F32, tag="ot", name="ot")
                if alt % 2 == 0:
                    nc.vector.tensor_copy(ot[:mc, :], pp[:mc, :])
                else:
                    nc.scalar.copy(ot[:mc, :], pp[:mc, :])
                nc.sync.dma_start(out_d[P * c : P * c + mc, vs], ot[:mc, :])

            MM = 0.52  # ns per moving column

            def x_cost(ci):
                return KE * max(41.0, MM * lcs[ci][1])

            fillers = deque()
            for ci in range(len(lcs)):
                for m in range(NM):
                    fillers.append((x_cost(ci), emit_x_unit, (m, ci)))
            proj_added = 0
            n_proj = 0

            def add_ready_projs(done_rows):
                nonlocal proj_added, n_proj
                while (proj_added + 1) * P <= done_rows or (
                    proj_added == nchunks - 1 and done_rows >= L
                ):
                    c = proj_added
                    for v in range(NV):
                        cost = KH * max(41.0, MM * VC)
                        fillers.append((cost, emit_proj, (c, v, n_proj)))
                        n_proj += 1
                    proj_added += 1

            def fill(budget_ns):
                spent = 0.0
                while fillers and spent < budget_ns:
                    cost, fn, args = fillers.popleft()
                    fn(*args)
                    spent += cost

            # ---- head: first 16 x-units ramp the PE & unblock rec0's chain --
            for m in range(NM):
                cost, fn, args = fillers.popleft()
                fn(*args)

            # ---- recurrence over packed steps ----
            c_prev = c0_sb
            for t, n in enumerate(n_ts):
                n = int(n)
                off = int(offs[t])
                if t == 0:
                    hT = h0_sb
                    hsl = slice(0, n)
                else:
                    hT = hsT
                    po = int(offs[t - 1])
                    hsl = slice(po, po + n)

                banks = []
                for j in range(4):
                    g = gpsp.tile([P, 4, P], F32, tag="g", name="g")
                    for m in range(4):
                        for k in range(KH):
                            nc.tensor.matmul(
                                g[:, m, :n],
                                whh_sb[:, k, 4 * j + m, :],
                                hT[:, k, hsl],
                                start=(k == 0), stop=(k == KH - 1),
                            )
                    banks.append(g)

                # gates = xWT + h-part; nonlinearities; c/h update
                acts = []
                for j, af in enumerate((AF.Sigmoid, AF.Sigmoid, AF.Tanh, AF.Sigmoid)):
                    tmp = wkp.tile([P, 4, P], F32, tag="wk")
                    nc.vector.tensor_add(
                        tmp[:, :, :n], banks[j][:, :, :n],
                        xWT[:, 4 * j : 4 * j + 4, off : off + n],
                    )
                    a = wkp.tile([P, 4, P], F32, tag="wk")
                    nc.scalar.activation(a[:, :, :n], tmp[:, :, :n], af)
                    acts.append(a)
                i_s, f_s, g_s, o_s = acts
                t1 = wkp.tile([P, 4, P], F32, tag="wk")
                t2 = wkp.tile([P, 4, P], F32, tag="wk")
                nc.vector.tensor_mul(t1[:, :, :n], i_s[:, :, :n], g_s[:, :, :n])
                nc.vector.tensor_mul(t2[:, :, :n], f_s[:, :, :n], c_prev[:, :, :n])
                c_new = ccp.tile([P, KH, P], F32, tag="c", name="cn")
                nc.vector.tensor_add(c_new[:, :, :n], t1[:, :, :n], t2[:, :, :n])
                tct = wkp.tile([P, 4, P], F32, tag="wk")
                nc.scalar.activation(tct[:, :, :n], c_new[:, :, :n], AF.Tanh)
                nc.vector.tensor_mul(
                    hsT[:, :, off : off + n], o_s[:, :, :n], tct[:, :, :n]
                )
                c_prev = c_new

                add_ready_projs(int(offs[t + 1]))
                if t + 1 < len(n_ts):
                    fill(FILL_NS)

            # ---- tail: flush remaining projection units ----
            add_ready_projs(L)
            while fillers:
                cost, fn, args = fillers.popleft()
                fn(*args)

    nc.compile()
    return nc


_prog_cache = {}


def _get_program(n_ts, use_bias, use_linb):
    key = (tuple(int(x) for x in n_ts), bool(use_bias), bool(use_linb))
    if key not in _prog_cache:
        _prog_cache[key] = build_program(n_ts, use_bias, use_linb)
    return _prog_cache[key]


def kernel(
    features,
    captions,
    lengths,
    h0,
    c0,
    embed_w,
    w_ih,
    w_hh,
    b_ih,
    b_hh,
    lin_w,
    lin_b,
    maxlen,
    _trace=False,
):
    features = np.asarray(features, np.float32)
    captions = np.asarray(captions)
    lengths = np.asarray(lengths)
    h0 = np.asarray(h0, np.float32)
    c0 = np.asarray(c0, np.float32)
    embed_w = np.asarray(embed_w, np.float32)
    w_ih = np.asarray(w_ih, np.float32)
    w_hh = np.asarray(w_hh, np.float32)
    b_ih = np.asarray(b_ih, np.float32)
    b_hh = np.asarray(b_hh, np.float32)
    lin_w = np.asarray(lin_w, np.float32)
    lin_b = np.asarray(lin_b, np.float32)
    maxlen = int(maxlen)
    batch = captions.shape[0]

    # Sort rows by descending length (stable). pack_padded_sequence requires
    # descending lengths, so perm is normally the identity; the permutation
    # fallback keeps us correct on arbitrary length order.
    ln = lengths.astype(np.int64)
    perm = np.argsort(-ln, kind="stable")
    identity_perm = bool(np.all(perm == np.arange(batch)))
    lns = ln[perm]

    n_ts = []
    for t in range(maxlen):
        n = int((lns > t).sum())
        if n == 0:
            break
        n_ts.append(n)
    L = int(sum(n_ts))
    offs = np.concatenate([[0], np.cumsum(n_ts)]).astype(int)

    # host prep: packed transposed input sequence xT [E, L]
    xs = np.empty((L, E), np.float32)
    for t, n in enumerate(n_ts):
        sel = perm[:n]
        if t == 0:
            xs[offs[t] : offs[t] + n] = features[sel]
        else:
            xs[offs[t] : offs[t] + n] = embed_w[captions[sel, t - 1]]
    xT = np.ascontiguousarray(xs.T).astype(BF)

    wihT = np.ascontiguousarray(w_ih.T).astype(BF)
    whhT = np.ascontiguousarray(w_hh.T).astype(BF)
    h0T = np.ascontiguousarray(h0[perm].T).astype(BF)
    c0T = np.ascontiguousarray(c0[perm].T)
    linT = np.ascontiguousarray(lin_w.T).astype(BF)
    bias2 = (b_ih + b_hh).astype(np.float32)
    use_bias = bool(np.any(bias2))
    use_linb = bool(np.any(lin_b))

    nc = _get_program(n_ts, use_bias, use_linb)

    in_maps = []
    for s in range(NCORES):
        m = {
            "xT": xT,
            "wih": wihT,
            "whh": whhT,
            "h0T": h0T,
            "c0T": c0T,
            "linT": np.ascontiguousarray(linT[:, VS * s : VS * (s + 1)]),
        }
        if use_bias:
            m["biaspm"] = np.ascontiguousarray(bias2.reshape(NM, P).T)
        if use_linb:
            m["linb"] = np.ascontiguousarray(
                lin_b[VS * s : VS * (s + 1)].reshape(1, VS).astype(BF)
            )
        in_maps.append(m)

    res = run_bass_kernel_spmd(
        nc, in_maps, core_ids=list(range(NCORES)), trace=_trace
    )
    out = np.concatenate([np.asarray(r["out"]) for r in res.results], axis=1)

    if not identity_perm:
        # map packed rows computed in sorted order back to original order
        src = np.empty(L, np.int64)
        pos = 0
        inv_pos = {}
        for t, n in enumerate(n_ts):
            for j in range(n):
                inv_pos[(t, int(perm[j]))] = offs[t] + j
        for t in range(maxlen):
            for i in np.nonzero(ln > t)[0]:
                src[pos] = inv_pos[(t, int(i))]
                pos += 1
        out = out[src]

    if _trace:
        return out, res
    return out
